# revision 53
# baseline (speedup 1.0000x reference)
"""Trainium2 Bass kernel for nn_AttentionModule (sparse_attention).

Computation (reference):
  q = tanh(einsum('hde,be->hbd', Query, x))          H=8 D=256 E=1536
  k = tanh(einsum('hdf,blf->hbld', Key, bank))       B=64 L=256 F=768
  s = einsum('hbld,hbd->hbl', k, q)  masked softmax over l
  out = LeakyReLU_0.4(einsum('hbl,blf->bhf', attn, bank))

Strategy: data-parallel over batch B across 8 NeuronCores (8 b's per core).

Sparsity: the mask zeroes ~half the L positions; masked positions receive
-1e8 bias so their softmax weight is ~0 and they contribute nothing to the
output.  Host prep COMPACTS bank per-b to its unmasked columns (a gather --
all FLOPs stay on device).  The batch rows are SORTED by unmasked count and
dealt round-robin to cores, so every core's b-pair `bp` holds rows from the
same count stratum; each stratum is padded only to its own width w[bp]
(exact stratum max; [113,128) rounds to 128 to keep bankT DMA rows >=512B).
This cuts the dominant k-matmul and the score matmul by
~L/avg(w) (~1.9x vs dense) instead of L/max_count.

Device pipeline per core (PE stream issued to stay gap-free):
  - k = tanh(KeyT^T @ bankT) head-outer (KeyT streams one head per ~5us of
    PE work; bankT/KeyT fp16 by default -- 0.05% quantization error).
  - q-heads (fp16 Query stream) run between the first k-heads; q is
    DVE-block-transposed into the zero-padded score lhsT (qz) -- no PE
    transposes anywhere.
  - scores: all (h,dc) accumulate into one [40, 2w] psum per b-pair (rows
    32*b2+h); fused masked softmax (one exp / reciprocal per pair) on
    ACT/DVE; attn DVE-block-transposed; emb = attn @ bank with
    normalize+LeakyReLU fused into one [40,384] Prelu per half.
  - tail: all four scores first, softmax split into stats (psum-slot
    releasing adds/max/exp) and exp-dependent trans (recip + transposes)
    issued a stage apart so the DVE FIFO never stalls; every emb runs at
    least one full softmax-chain latency behind its score.
"""

import os
import numpy as np
import concourse.bass as bass  # noqa: F401
import concourse.mybir as mybir
import concourse.tile as tile
from concourse import bacc, bass_utils

F32 = mybir.dt.float32
F32R = mybir.dt.float32r
FP16 = mybir.dt.float16
AF = mybir.ActivationFunctionType
AX = mybir.AxisListType

# dtype of the big k-matmul operands (KeyT / bankT). fp16 halves their DMA
# at ~0.05% quantization error; fp32r keeps tf32-grade accuracy.
KF16 = os.environ.get("KERNEL_KF16", "1") == "1"

H, D, E, F = 8, 256, 1536, 768
B, L = 64, 256
NCORES = 8
BPC = B // NCORES          # 8 b's per core
NBP = BPC // 2             # 4 b-pairs per core
EC, FC, DC = E // 128, F // 128, D // 128   # 12, 6, 2


def _build_program(widths, kf16):
    KMM = FP16 if kf16 else F32R
    W = list(widths)                    # per-b-pair compacted length
    N2 = [2 * w for w in W]             # k / score moving width per pair
    LPT = [-(-w // 32) * 32 for w in W]  # padded to DVE 32x32 transpose grid
    NBLK = [lpt // 32 for lpt in LPT]
    LREM = [max(0, w - 128) for w in W]  # l rows beyond the first 128
    WMAX, N2MAX, LPTMAX = max(W), max(N2), max(LPT)
    LREMMAX = max(LREM)
    USE_MB = True   # bias-free variant measured slower (two serial exps)

    nc = bacc.Bacc("TRN2", target_bir_lowering=False, debug=False,
                   enable_asserts=False, num_devices=NCORES)
    qt = nc.dram_tensor("qt", [H, E, D], FP16, kind="ExternalInput").ap()
    kt = nc.dram_tensor("kt", [H, F, D], KMM, kind="ExternalInput").ap()
    bkt = [nc.dram_tensor(f"bkt{bp}", [F, N2[bp]], KMM,
                          kind="ExternalInput").ap() for bp in range(NBP)]
    bkn = [nc.dram_tensor(f"bkn{bp}", [2, W[bp], F], FP16,
                          kind="ExternalInput").ap() for bp in range(NBP)]
    xt = nc.dram_tensor("xt", [128, EC * BPC], FP16, kind="ExternalInput").ap()
    mb = [nc.dram_tensor(f"mb{bp}", [2, H, W[bp]], F32,
                         kind="ExternalInput").ap() for bp in range(NBP)]
    out = nc.dram_tensor("out", [BPC, H, F], F32, kind="ExternalOutput").ap()

    with tile.TileContext(nc) as tc:
        with tc.tile_pool(name="const", bufs=1) as cpool, \
             tc.tile_pool(name="weights", bufs=1) as wpool, \
             tc.tile_pool(name="stream", bufs=4) as spool, \
             tc.tile_pool(name="small", bufs=4) as smpool, \
             tc.tile_pool(name="psK", bufs=3, space="PSUM") as psK, \
             tc.tile_pool(name="psS", bufs=2, space="PSUM") as psS, \
             tc.tile_pool(name="psM", bufs=3, space="PSUM") as psM:

            # ---------------- resident SBUF tiles ------------------------
            kt_tiles = [wpool.tile([128, FC * D], KMM, name=f"kt_sb{h}",
                                   tag=f"kt_sb{h}") for h in range(H)]
            # bankT per pair: [128(f), fc, (b2 l)]
            bktA = [cpool.tile([128, FC * N2[bp]], KMM, name=f"bktA{bp}")
                    for bp in range(NBP)]
            bktA_v = [bktA[bp][:].rearrange("p (fc n) -> p fc n", fc=FC)
                      for bp in range(NBP)]
            # bank (natural layout), emb rhs: first 128 l-rows + remainder
            bkn0 = cpool.tile([128, BPC * F], FP16, name="bkn0")
            bkn1 = (cpool.tile([max(32, LREMMAX), BPC * F], FP16, name="bkn1")
                    if LREMMAX else None)
            xt_sb = cpool.tile([128, EC * BPC], FP16)
            mbA = [cpool.tile([8, 2 * W[bp]], F32, name=f"mbA{bp}")
                   for bp in range(NBP)]
            # zero-padded score lhsT: col = bp*640 + (2h+dc)*40 + 32*b2 + h
            qz = cpool.tile([128, NBP * 640], FP16)
            qz_v = qz[:].rearrange("p (bp blk c) -> p bp blk c", bp=NBP, blk=16)
            # k = tanh(...), resident per pair: [128(d), h, dc, n2]
            k_t = [cpool.tile([128, H * DC * N2[bp]], FP16, name=f"k_t{bp}")
                   for bp in range(NBP)]
            k_v = [k_t[bp][:].rearrange("p (h dc n) -> p h dc n", h=H, dc=DC)
                   for bp in range(NBP)]

            # ---------------- DMA issue helpers --------------------------
            def load_kt_piece(h, piece, pieces):
                fc_per = FC // pieces
                nc.sync.dma_start(
                    kt_tiles[h][:, piece * fc_per * D:(piece + 1) * fc_per * D]
                    .rearrange("p (fc d) -> p fc d", fc=fc_per),
                    kt[h, piece * fc_per * 128:(piece + 1) * fc_per * 128]
                    .rearrange("(fc p) d -> p fc d", p=128))

            def load_kt(h, pieces=1):
                for piece in range(pieces):
                    load_kt_piece(h, piece, pieces)

            def load_bktA(bp, pieces=1):
                fc_per = FC // pieces
                for piece in range(pieces):
                    nc.sync.dma_start(
                        bktA_v[bp][:, piece * fc_per:(piece + 1) * fc_per],
                        bkt[bp][piece * fc_per * 128:(piece + 1) * fc_per * 128]
                        .rearrange("(fc p) n -> p fc n", p=128))

            def load_mb(bp):
                nc.sync.dma_start(
                    mbA[bp][:].rearrange("h (b l) -> h b l", b=2),
                    mb[bp].rearrange("b h l -> h b l"))

            def load_bkn():
                for bp in range(NBP):
                    for b2 in range(2):
                        b = 2 * bp + b2
                        nc.sync.dma_start(
                            bkn0[0:min(W[bp], 128), b * F:(b + 1) * F],
                            bkn[bp][b2, 0:min(W[bp], 128)])
                        if LREM[bp]:
                            nc.sync.dma_start(
                                bkn1[0:LREM[bp], b * F:(b + 1) * F],
                                bkn[bp][b2, 128:128 + LREM[bp]])

            # ---------------- q phase (fp16) ------------------------------
            # q = tanh(x @ Query^T); two stacks of 4 heads (rows 32*hh, 8
            # live rows each -- engine writes need 32-aligned partition
            # bases) -> DVE 32x32 block transposes -> strided copies into qz.
            q_stacks = [smpool.tile([128, D], FP16, name=f"qs{g}", tag=f"qs{g}")
                        for g in range(2)]

            def q_head(h):
                g, hh = divmod(h, 4)
                pq = psM.tile([BPC, D], F32, name="pq", tag="psm")
                qt_c = spool.tile([128, EC * D], FP16, name="qt_c", tag="qt_c")
                nc.sync.dma_start(
                    qt_c[:].rearrange("p (ec d) -> p ec d", ec=EC),
                    qt[h].rearrange("(ec p) d -> p ec d", p=128))
                for ec in range(EC):
                    nc.tensor.matmul(pq[:], xt_sb[:, ec * BPC:(ec + 1) * BPC],
                                     qt_c[:, ec * D:(ec + 1) * D],
                                     start=(ec == 0), stop=(ec == EC - 1))
                nc.scalar.activation(q_stacks[g][32 * hh:32 * hh + 8, :], pq[:],
                                     AF.Tanh)

            def q_scatter(g):
                for dc in range(DC):
                    qT = smpool.tile([128, 128], FP16, name=f"qT{g}{dc}",
                                     tag="qT")
                    for i in range(4):          # head row-blocks
                        for j in range(4):      # d sub-blocks
                            nc.vector.transpose(
                                qT[32 * j:32 * j + 32, 32 * i:32 * i + 32],
                                q_stacks[g][32 * i:32 * i + 32,
                                            dc * 128 + 32 * j:dc * 128 + 32 * j + 32])
                    # qT col = 32*hh + 2*bp + b2 (b = 2bp+b2 local batch)
                    qT_v = qT[:].rearrange("p (hh bpx b2) -> p hh bpx b2",
                                           hh=4, bpx=16)
                    for hh in range(4):
                        h = 4 * g + hh
                        for b2 in range(2):
                            nc.vector.tensor_copy(
                                qz_v[:, :, 2 * h + dc, 32 * b2 + h],
                                qT_v[:, hh, 0:4, b2])

            # ---------------- k phase -------------------------------------
            def k_head(h, bps=range(NBP)):
                for bp in bps:
                    for dc in range(DC):
                        pk = psK.tile([128, N2MAX], F32, name="pk", tag="pk")
                        for fc in range(FC):
                            nc.tensor.matmul(
                                pk[:, 0:N2[bp]],
                                kt_tiles[h][:, fc * D + dc * 128:
                                            fc * D + dc * 128 + 128],
                                bktA_v[bp][:, fc],
                                start=(fc == 0), stop=(fc == FC - 1))
                        nc.scalar.activation(k_v[bp][:, h, dc],
                                             pk[:, 0:N2[bp]], AF.Tanh)

            # ---------------- score / softmax / emb -----------------------
            simsafe = os.environ.get("KERNEL_SIM_SAFE", "0") == "1"

            def score_mms(bp):
                ps40 = psS.tile([40, N2MAX], F32, name="ps40", tag="ps40")
                for h in range(H):
                    for dc in range(DC):
                        nc.tensor.matmul(
                            ps40[:, 0:N2[bp]], qz_v[:, bp, 2 * h + dc],
                            k_v[bp][:, h, dc],
                            start=(h == 0 and dc == 0),
                            stop=(h == H - 1 and dc == DC - 1))
                return ps40

            def softmax_stats(bp, ps40):
                # both b2 stacked at rows 0 / 32 of p40: one reciprocal and
                # one Prelu scale vector per b-pair.  In compacted mode the
                # padded slots carry exactly-zero bank columns (k = tanh(0)
                # = 0 -> score 0 << max; bkn pad rows are zero), so no mask
                # bias is needed and max/exp read the score psum directly.
                w = W[bp]
                nm40 = smpool.tile([40, 1], F32, name="nm40", tag="nm40")
                zs40 = smpool.tile([40, 1], F32, name="zs40", tag="zs40")
                p40 = smpool.tile([64, LPTMAX], FP16, name="p40", tag="p40")
                if USE_MB:
                    s40 = smpool.tile([40, WMAX], F32, name="s40", tag="s40")
                    for b2 in range(2):
                        nc.vector.tensor_add(s40[32 * b2:32 * b2 + 8, 0:w],
                                             ps40[32 * b2:32 * b2 + 8,
                                                  w * b2:w * b2 + w],
                                             mbA[bp][:, w * b2:w * b2 + w])
                        nc.vector.reduce_max(nm40[32 * b2:32 * b2 + 8, :],
                                             s40[32 * b2:32 * b2 + 8, 0:w],
                                             axis=AX.X, negate=True)
                    nc.scalar.activation(p40[0:40, 0:w], s40[0:40, 0:w],
                                         AF.Exp, bias=nm40[:],
                                         accum_out=zs40[:])
                else:
                    for b2 in range(2):
                        nc.vector.reduce_max(nm40[32 * b2:32 * b2 + 8, :],
                                             ps40[32 * b2:32 * b2 + 8,
                                                  w * b2:w * b2 + w],
                                             axis=AX.X, negate=True)
                    for b2 in range(2):
                        nc.scalar.activation(
                            p40[32 * b2:32 * b2 + 8, 0:w],
                            ps40[32 * b2:32 * b2 + 8, w * b2:w * b2 + w],
                            AF.Exp, bias=nm40[32 * b2:32 * b2 + 8, :],
                            accum_out=zs40[32 * b2:32 * b2 + 8, :])
                return zs40, p40

            def softmax_trans(bp, zs40, p40):
                rz40 = smpool.tile([40, 1], F32, name="rz40", tag="rz40")
                nc.vector.reciprocal(rz40[:], zs40[:])
                pT0 = smpool.tile([128, 64], FP16, name="pT0", tag="pT0")
                pT1 = (smpool.tile([max(32, LPTMAX - 128), 64], FP16,
                                   name="pT1", tag="pT1")
                       if LREMMAX else None)
                for i in range(2):              # b2 row-blocks
                    for j in range(NBLK[bp]):
                        row = 32 * j
                        dst = (pT0[row:row + 32, 32 * i:32 * i + 32]
                               if row < 128
                               else pT1[row - 128:row - 96, 32 * i:32 * i + 32])
                        nc.vector.transpose(
                            dst, p40[32 * i:32 * i + 32, row:row + 32])
                return rz40, pT0, pT1

            def emb(bp, rz40, pT0, pT1):
                # alternate emb psums across both pools: the score ring is
                # idle by now, so this doubles the Prelu-drain slack
                o40 = smpool.tile([40, F], F32, name="o40", tag="o40")
                for fh in range(2):
                    pe = (psM.tile([40, 384], F32, name="pe", tag="psm")
                          if fh == 0 else
                          psS.tile([40, 384], F32, name="pe2", tag="ps40"))
                    for b2 in range(2):
                        b = 2 * bp + b2
                        nc.tensor.matmul(
                            pe[32 * b2:32 * b2 + 8, :],
                            pT0[0:min(W[bp], 128), 32 * b2:32 * b2 + 8],
                            bkn0[0:min(W[bp], 128),
                                 b * F + fh * 384:b * F + fh * 384 + 384],
                            start=True, stop=(LREM[bp] == 0))
                        if LREM[bp]:
                            nc.tensor.matmul(
                                pe[32 * b2:32 * b2 + 8, :],
                                pT1[0:LREM[bp], 32 * b2:32 * b2 + 8],
                                bkn1[0:LREM[bp],
                                     b * F + fh * 384:b * F + fh * 384 + 384],
                                start=False, stop=True)
                    if simsafe:
                        nc.scalar.activation(o40[:, fh * 384:fh * 384 + 384],
                                             pe[0:40, :], AF.Copy,
                                             scale=rz40[:])
                    else:
                        nc.scalar.activation(o40[:, fh * 384:fh * 384 + 384],
                                             pe[0:40, :], AF.Prelu,
                                             scale=rz40[:], alpha=0.4)
                for b2 in range(2):
                    nc.sync.dma_start(out[2 * bp + b2],
                                      o40[32 * b2:32 * b2 + 8, :])

            # ---------------- program order -------------------------------
            # Front-load the high-leverage DMAs: each kt[h] unlocks ~5us of
            # PE work, bankT is shared by all heads.  The Query stream (1:1
            # DMA:PE) queues behind them and is consumed mid-sweep.
            nc.vector.memset(qz[:], 0.0)
            for piece in range(2):
                load_kt_piece(0, piece, 2)
                nc.sync.dma_start(
                    bktA_v[0][:, piece * 3:piece * 3 + 3],
                    bkt[0][piece * 384:piece * 384 + 384]
                    .rearrange("(fc p) n -> p fc n", p=128))
            load_bktA(1)
            load_bktA(2)
            load_bktA(3)
            load_kt(1)
            load_kt(2)
            load_kt(3)
            nc.sync.dma_start(xt_sb[:], xt)
            k_head(0)
            k_head(1)
            k_head(2)
            for h in range(4):
                q_head(h)
            q_scatter(0)
            k_head(3)
            for h in range(4, H):
                q_head(h)
            q_scatter(1)
            load_kt(4)
            k_head(4)
            load_kt(5)
            if USE_MB:
                for bp in range(NBP):
                    load_mb(bp)
            k_head(5)
            load_kt(6)
            load_bkn()
            k_head(6)
            load_kt(7)
            k_head(7)

            # scores with the heaviest pair (bp0) first: by the time the PE
            # reaches each emb, its softmax chain finished several stages ago
            ps0 = score_mms(0)
            st0 = softmax_stats(0, ps0)
            ps1 = score_mms(1)
            st1 = softmax_stats(1, ps1)
            tr0 = softmax_trans(0, *st0)
            ps2 = score_mms(2)
            st2 = softmax_stats(2, ps2)
            tr1 = softmax_trans(1, *st1)
            ps3 = score_mms(3)
            st3 = softmax_stats(3, ps3)
            tr2 = softmax_trans(2, *st2)
            emb(0, *tr0)
            emb(1, *tr1)
            tr3 = softmax_trans(3, *st3)
            emb(2, *tr2)
            emb(3, *tr3)

    nc.finalize()
    return nc


def _plan(mask):
    """Sort batch rows by unmasked count, deal round-robin to cores.

    Returns (order, widths): order[8*slot + core] = original b index;
    widths[bp] = padded length for slot pair bp (same on every core).
    """
    counts = np.asarray(mask).astype(bool).sum(axis=1)
    order = np.argsort(-counts, kind="stable")
    if counts.min() == 0:
        return order, (L, L, L, L)
    widths = []
    for bp in range(NBP):
        stratum = counts[order[16 * bp:16 * bp + 16]]
        w = int(min(L, max(32, int(stratum.max()))))
        if 113 <= w < 128:
            w = 128   # keep bankT DMA rows >= 512B (2x penalty below)
        widths.append(w)
    return order, tuple(widths)


def _host_prep(x, bank, mask, Query, Key, order, widths, kf16):
    x = np.ascontiguousarray(x, dtype=np.float32)
    bank = np.ascontiguousarray(bank, dtype=np.float32)
    Query = np.ascontiguousarray(Query, dtype=np.float32)
    Key = np.ascontiguousarray(Key, dtype=np.float32)
    mask = np.asarray(mask)

    kdt = np.float16 if kf16 else np.float32
    qt = np.ascontiguousarray(Query.transpose(0, 2, 1)).astype(np.float16)
    kt = np.ascontiguousarray(Key.transpose(0, 2, 1)).astype(kdt)  # [H, F, D]

    in_maps = []
    for c in range(NCORES):
        bs = [int(order[8 * s + c]) for s in range(BPC)]   # slot -> b
        m = {"qt": qt, "kt": kt}
        xs = x[bs]                                         # [BPC, E]
        m["xt"] = np.ascontiguousarray(
            xs.T.reshape(EC, 128, BPC).transpose(1, 0, 2)
            .reshape(128, EC * BPC)).astype(np.float16)
        for bp in range(NBP):
            w = widths[bp]
            bc = np.zeros((2, w, F), dtype=np.float32)
            bct = np.zeros((2, w, F), dtype=np.float32)
            mbias = np.full((2, w), np.float32(-1e8))
            for b2 in range(2):
                b = bs[2 * bp + b2]
                nz = np.flatnonzero(mask[b])[:w]
                if len(nz) == 0:
                    # all-masked row: reference replaces every score with
                    # -1e8 (uniform softmax over ALL L slots; widths are L
                    # in this fallback).  Zero bank into the k-matmul so the
                    # scores are constant; emb still uses the real bank.
                    bc[b2, :min(w, L)] = bank[b, :min(w, L)]
                    continue
                bc[b2, :len(nz)] = bank[b, nz]
                bct[b2, :len(nz)] = bank[b, nz]
                mbias[b2, :len(nz)] = 0.0
            m[f"bkt{bp}"] = np.ascontiguousarray(
                bct.transpose(2, 0, 1).reshape(F, 2 * w)).astype(kdt)
            m[f"bkn{bp}"] = np.ascontiguousarray(bc).astype(np.float16)
            m[f"mb{bp}"] = np.ascontiguousarray(
                np.repeat(mbias[:, None, :], H, axis=1)).astype(np.float32)
        in_maps.append(m)
    return in_maps


_NC_CACHE = {}


def kernel(x, bank, mask, Query, Key):
    order, widths = _plan(mask)
    key = (widths, KF16)
    if key not in _NC_CACHE:
        _NC_CACHE[key] = _build_program(widths, KF16)
    nc = _NC_CACHE[key]
    in_maps = _host_prep(x, bank, mask, Query, Key, order, widths, KF16)

    trace = os.environ.get("KERNEL_TRACE", "0") == "1"
    res = bass_utils.run_bass_kernel_spmd(nc, in_maps,
                                          core_ids=list(range(NCORES)),
                                          trace=trace)
    if trace:
        print("exec_time_ns:", res.exec_time_ns,
              "mean:", res.mean_exec_time_ns,
              "core:", res.max_exec_time_core_id)
    full = np.empty((B, H, F), dtype=np.float32)
    for c in range(NCORES):
        o = res.results[c]["out"]
        for s in range(BPC):
            full[int(order[8 * s + c])] = o[s]
    return full
